# revision 3
# baseline (speedup 1.0000x reference)
"""Trainium2 Bass kernel for nn_CausalConvolution.

Reference computation (B=16, H=4, S=8, W=256, F=16):
    stacked[h,x,y,j,i] = kernel[h,x,y,(i-j-1)%W] * (i<=j)        # [H,S,S,W,W]
    out[b,h,x,y,j,f]   = sum_i stacked[h,x,y,j,i] * x[b,x,i,f]   # einsum
    out                = out / (j+1)
    diag (x==y): out[...,j,:] = out[...,j-1,:]/  (roll by 1), 0 at j=0

Key identities used here:
  * stacked[h,x,y,j,i] = Pz[255 + i - j], where Pz = concat(kernel[h,x,y,:],
    zeros(128)).  The triangular mask falls out of the zero padding.
  * A single DMA with an overlapping "sliding window" access pattern
    materializes Wt[i,u] = Pz[i+u] (= stacked column j=255-u) in SBUF.
  * The diagonal (x==y) roll-by-one is equivalent to using a shifted kernel
    vector Pz'[t] = Pz[t+1] and scale 1/j instead of 1/(j+1) -- pure host-side
    data prep, so the device program is identical for every pair and core.

Sharding: x (axis 2, size 8) across the 8 NeuronCores.  Per core: 32 (h,y)
pairs; per pair 3 fp32r matmuls (N=256, full PE rate) accumulate
psum[p, b*f] = conv column j (reversed: j=127-p / 255-p), then a per-partition
scaled copy PSUM->SBUF fuses the 1/(j+1) division, and one 256KB contiguous
DMA writes the pair's slab.  Host un-reverses j and re-permutes axes.
"""

import sys

for _p in ("/opt/trn_rl_repo", "/root/.axon_site/_ro/trn_rl_repo"):
    if _p not in sys.path:
        sys.path.append(_p)

import numpy as np

import concourse.bass as bass
import concourse.bacc as bacc
import concourse.mybir as mybir
import concourse.tile as tile
from concourse.bass_utils import run_bass_kernel_spmd

B, H, S, W, F = 16, 4, 8, 256, 16
NCORES = 8
NPAIR = H * S  # 32 (h,y) pairs per core
KPADLEN = W + 128  # 384
f32 = mybir.dt.float32
f32r = mybir.dt.float32r

_CACHE = {}


def _build_nc():
    """Build the (SPMD-identical) Bass program for one core."""
    nc = bacc.Bacc("TRN2", target_bir_lowering=False, debug=False,
                   num_devices=NCORES)

    # xt: x[:, c].transpose(1,0,2).reshape(W, B*F)  -- [i, b*f]
    xt = nc.dram_tensor("xt", [W, B * F], f32r, kind="ExternalInput")
    # kpad row per pair: [kernel_vec(256) | zeros(128)]  (diag pair shifted)
    kpad = nc.dram_tensor("kpad", [NPAIR, KPADLEN], f32r, kind="ExternalInput")
    # scales: per pair p, column 2p = psum0 scale, 2p+1 = psum1 scale
    scales = nc.dram_tensor("scales", [128, 2 * NPAIR], f32, kind="ExternalInput")
    # out[p, jj, m*256 + bf] ; final j = (m+1)*128 - 1 - jj
    out = nc.dram_tensor("out", [NPAIR, 128, 512], f32, kind="ExternalOutput")

    with tile.TileContext(nc) as tc:
        with (
            tc.tile_pool(name="xp", bufs=1) as xp,
            tc.tile_pool(name="scp", bufs=1) as scp,
            tc.tile_pool(name="wtp", bufs=8) as wtp,
            tc.tile_pool(name="obp", bufs=6) as obp,
            tc.tile_pool(name="psp", bufs=4, space="PSUM") as psp,
        ):
            x0 = xp.tile([128, 256], f32r, tag="x0")
            x1 = xp.tile([128, 256], f32r, tag="x1")
            nc.sync.dma_start(x0[:], xt[0:128, :])
            nc.sync.dma_start(x1[:], xt[128:256, :])
            sc = scp.tile([128, 2 * NPAIR], f32)
            nc.sync.dma_start(sc[:], scales[:])

            for p in range(NPAIR):
                wt = wtp.tile([128, 256], f32r)
                # sliding window: wt[i, u] = kpad[p, i + u]
                src = bass.AP(kpad, p * KPADLEN, [[1, 128], [1, 256]])
                nc.sync.dma_start(wt[:], src)

                ps0 = psp.tile([128, 256], f32, tag="ps0")
                ps1 = psp.tile([128, 256], f32, tag="ps1")
                # psum0[p', bf] = conv col j=127-p'   (i in [0,128) only)
                nc.tensor.matmul(ps0[:], wt[:, 128:256], x0[:],
                                 start=True, stop=True)
                # psum1[p', bf] = conv col j=255-p'   (i in [0,128) + [128,256))
                nc.tensor.matmul(ps1[:], wt[:, 0:128], x0[:],
                                 start=True, stop=False)
                nc.tensor.matmul(ps1[:], wt[:, 128:256], x1[:],
                                 start=False, stop=True)

                ob = obp.tile([128, 512], f32)
                nc.vector.tensor_scalar_mul(ob[:, 0:256], ps0[:],
                                            sc[:, 2 * p:2 * p + 1])
                nc.scalar.mul(ob[:, 256:512], ps1[:],
                              sc[:, 2 * p + 1:2 * p + 2])
                nc.sync.dma_start(out[p], ob[:])

    nc.compile()
    return nc


def _host_inputs(x, kern):
    """Per-core input maps (host-side data prep)."""
    in_maps = []
    q = np.arange(128)
    # off-diagonal scales
    s0 = 1.0 / (128.0 - q)
    s1 = 1.0 / (256.0 - q)
    # diagonal-pair scales (kernel vector shifted by one, scale 1/j)
    d0 = np.where(q == 127, 0.0, 1.0 / np.maximum(127.0 - q, 1.0))
    d1 = 1.0 / (255.0 - q)
    for c in range(NCORES):
        xt = np.ascontiguousarray(
            x[:, c].transpose(1, 0, 2).reshape(W, B * F), dtype=np.float32)
        kp = np.zeros((NPAIR, KPADLEN), np.float32)
        sc = np.empty((128, 2 * NPAIR), np.float32)
        for h in range(H):
            for y in range(S):
                p = h * S + y
                if y == c:
                    kp[p, 0:W - 1] = kern[h, c, y, 1:W]
                    sc[:, 2 * p] = d0
                    sc[:, 2 * p + 1] = d1
                else:
                    kp[p, 0:W] = kern[h, c, y, :]
                    sc[:, 2 * p] = s0
                    sc[:, 2 * p + 1] = s1
        in_maps.append({"xt": xt, "kpad": kp, "scales": sc})
    return in_maps


def _assemble(results):
    """Per-core [32,128,512] slabs -> full [B,H,S,S,W,F] output."""
    outs = []
    for c in range(NCORES):
        o = results[c]["out"].reshape(H, S, 128, 2, B, F)  # [h,y,jj,m,b,f]
        o = o[:, :, ::-1]                   # jj' = 127-jj  ->  j = m*128+jj'
        o = o.transpose(4, 0, 1, 3, 2, 5)   # [b,h,y,m,jj',f]
        outs.append(o.reshape(B, H, S, W, F))
    return np.ascontiguousarray(np.stack(outs, axis=2))


def _run(x, kern, **spmd_kwargs):
    if "nc" not in _CACHE:
        _CACHE["nc"] = _build_nc()
    nc = _CACHE["nc"]
    in_maps = _host_inputs(np.asarray(x, np.float32), np.asarray(kern, np.float32))
    res = run_bass_kernel_spmd(nc, in_maps, core_ids=list(range(NCORES)),
                               **spmd_kwargs)
    return _assemble(res.results), res


def kernel(x, kernel):
    out, _ = _run(x, kernel)
    return out
